# revision 1
# baseline (speedup 1.0000x reference)
"""Trainium2 Bass kernel for a single causal attention head.

  q = x @ Wq.T; k = pos_emb @ Wk.T; v = x @ Wv.T
  out = softmax(causal(q @ k.T / sqrt(E))) @ v

Sharding (8 cores): core c -> (batch b = c//2, half h = c%2). Core h owns the
interleaved 128-row blocks {2j+h} of batch b (queries AND keys) so causal work
is balanced across the pair. Each core projects Q/K/V for its own rows, the
pair AllGathers K/V, then each core runs attention for its own queries over all
keys. All activations are fed host-transposed ([E, T_core]) so every matmul has
the contraction dim on partitions with zero on-chip activation transposes.

Attention uses the transposed-scores layout: s^T[k, q] tiles so softmax
denominators come from a ones-vector matmul (partition reduction on the PE) and
the attn @ v matmul consumes exp tiles directly (no transposes). exp is applied
on the scalar engine straight out of PSUM with the 1/sqrt(E) scale fused. The
program is rank-uniform: causal boundary behaviour is data (per-core mask
tiles), not control flow.
"""

import os
import sys
from contextlib import ExitStack
from dataclasses import dataclass

import numpy as np


def _ensure_path():
    try:
        import concourse.bass  # noqa: F401
    except ImportError:
        for p in ("/opt/trn_rl_repo", "/root/.axon_site/_ro/trn_rl_repo"):
            if os.path.isdir(p) and p not in sys.path:
                sys.path.insert(0, p)


_ensure_path()

import concourse.bass as bass  # noqa: E402
import concourse.mybir as mybir  # noqa: E402
import concourse.tile as tile  # noqa: E402
from concourse.masks import make_identity  # noqa: E402

P = 128
F16 = mybir.dt.float16
F32 = mybir.dt.float32


@dataclass(frozen=True)
class Cfg:
    B: int = 4
    T: int = 2048
    E: int = 4096
    H: int = 128
    QGB: int = 4  # 128-blocks per query group (matmul free dim = QGB*P)

    @property
    def NB(self):  # key/query 128-blocks per core
        return self.T // (2 * P)

    @property
    def TB(self):  # rows per core
        return self.NB * P

    @property
    def NQG(self):  # query groups per core
        return self.NB // self.QGB

    @property
    def QG(self):  # queries per group
        return self.QGB * P

    @property
    def EC(self):  # contraction chunks
        return self.E // P


FULL = Cfg()

# walrus CoreV3 setupSyncWait rejects instructions carrying more than
# MAX_SYNC_WAITS wait conditions; Tile's kernel-tail drain (and occasionally a
# body instruction) can exceed it. Excess waits are hoisted onto injected
# same-engine NoOp instructions placed immediately before the offender, which
# preserves semantics (the sequencer stalls at the carrier first).
MAX_SYNC_WAITS = 1


def _dedupe_ldweights(nc: bass.Bass):
    """Drop PE Ldweights whose stationary operand is identical to the weights
    already loaded (e.g. the two 512-column halves of one projection chunk).
    Any sync conditions move onto the following PE instruction."""
    import orjson

    n = 0
    for fn in nc.m.functions:
        for bb in fn.blocks:
            out = []
            last_sig = None
            pending_sync = None
            for inst in bb.instructions:
                if getattr(inst, "engine", None) != mybir.EngineType.PE:
                    out.append(inst)
                    continue
                d = orjson.loads(nc.instruction_to_json(inst))
                if d["opcode"] == "Ldweights":
                    sig = orjson.dumps(
                        [d.get("ins"), d.get("tile_position"), d.get("tile_size")]
                    )
                    if sig == last_sig:
                        si = inst.sync_info
                        if si and (si.on_wait or si.on_update):
                            pending_sync = si
                        n += 1
                        continue  # drop
                    last_sig = sig
                if pending_sync is not None:
                    si = inst.sync_info
                    if si is None:
                        inst.sync_info = pending_sync
                    else:
                        si.on_wait = list(pending_sync.on_wait) + list(si.on_wait)
                        si.on_update = list(pending_sync.on_update) + list(
                            si.on_update
                        )
                    pending_sync = None
                out.append(inst)
            assert pending_sync is None
            bb.instructions[:] = out
    return n


def _split_sync_waits(nc: bass.Bass, maxw: int = MAX_SYNC_WAITS):
    n = 0
    for fn in nc.m.functions:
        for bb in fn.blocks:
            out = []
            for inst in bb.instructions:
                si = inst.sync_info
                waits = list(si.on_wait) if si and si.on_wait else []
                if len(waits) > maxw:
                    excess, keep = waits[:-maxw], waits[-maxw:]
                    for k in range(0, len(excess), maxw):
                        carrier = mybir.InstNoOp(
                            name=f"{inst.name}-wsplit{n}",
                            engine=inst.engine,
                            ins=[],
                            outs=[],
                            sync_info=mybir.SyncInfo(
                                on_wait=excess[k : k + maxw], on_update=[]
                            ),
                        )
                        n += 1
                        out.append(carrier)
                    si.on_wait = keep
                out.append(inst)
            bb.instructions[:] = out
    return n


def build(cfg: Cfg, mock_cc: bool = False, reps: int = 1) -> bass.Bass:
    assert cfg.H == P
    TB, NB, EC, QG, QGB, NQG, H = (
        cfg.TB, cfg.NB, cfg.EC, cfg.QG, cfg.QGB, cfg.NQG, cfg.H,
    )
    KV = TB * H  # fp16 elements of one of {kT, v} local halves

    nc = bass.Bass("TRN2", target_bir_lowering=False, debug=False, num_devices=8)

    xT = nc.dram_tensor("xT", [cfg.E, TB], F16, kind="ExternalInput").ap()
    peT = nc.dram_tensor("peT", [cfg.E, TB], F16, kind="ExternalInput").ap()
    # per-chunk interleave [wk_e | wv_e | wq_e], one streaming slice per
    # super-chunk rides ahead of the peT stream on the sync queue
    w_all = nc.dram_tensor("w_all", [P, EC * 3 * H], F16, kind="ExternalInput").ap()
    qmask = nc.dram_tensor("qmask", [P, 2 * P], F16, kind="ExternalInput").ap()
    outT = nc.dram_tensor("outT", [H, TB], F32, kind="ExternalOutput").ap()

    cc_in = nc.dram_tensor("cc_in", [2 * KV], F16).ap()
    cc_k_out = nc.dram_tensor("cc_k_out", [2, KV], F16).ap()
    cc_v_out = nc.dram_tensor("cc_v_out", [2, KV], F16).ap()

    scale = 1.0 / np.sqrt(float(cfg.E))

    with tile.TileContext(nc) as tc, ExitStack() as ctx:
        if reps > 1:  # timing amplification harness (not used for grading)
            ctx.enter_context(tc.For_i(0, reps, 1))
        consts = ctx.enter_context(tc.tile_pool(name="consts", bufs=1))
        big = ctx.enter_context(tc.tile_pool(name="big", bufs=1))
        pe_pool = ctx.enter_context(tc.tile_pool(name="pe", bufs=3))
        x_pool = ctx.enter_context(tc.tile_pool(name="xp", bufs=5))
        e_pool = ctx.enter_context(tc.tile_pool(name="eT", bufs=2 * QGB * NQG * NQG + 6))
        sm = ctx.enter_context(tc.tile_pool(name="sm", bufs=2))

        proj_ctx = ExitStack()
        pp = proj_ctx.enter_context(tc.tile_pool(name="pp", bufs=6, space="PSUM"))
        tr_ps_pool = proj_ctx.enter_context(
            tc.tile_pool(name="trp", bufs=2, space="PSUM")
        )

        # ---- constants ----
        # DMA queue plan: peT stream + W slices + all post-belt bounce/readback
        # and output traffic ride the sync (SP) HWDGE queue; the xT stream and
        # masks ride the scalar (ACT) HWDGE queue, which must drain before the
        # attention exps issue — so nothing post-belt goes there.
        ones_col = consts.tile([P, 1], F16, tag="ones_col")
        nc.any.memset(ones_col[:], 1.0)
        ones_row = consts.tile([1, P], F32, tag="ones_row")
        nc.any.memset(ones_row[:], 1.0)
        ident = consts.tile([P, P], F16, tag="ident")
        make_identity(nc, ident[:])
        # preload the ACT Exp function table during the DMA belt so the first
        # attention exp doesn't pay the cold-table load on the critical path
        warm = consts.tile([P, 1], F16, tag="warm")
        nc.scalar.activation(
            warm[:], ones_col[:], mybir.ActivationFunctionType.Exp
        )

        NT = TB // 512 if TB >= 512 else 1
        FD = min(512, TB)  # projection matmul free dim

        # variable super-chunks: small head so the PE starts almost
        # immediately, 4-chunk body for DMA efficiency
        SCS = []
        rem = EC
        for want in [1, 1, 2] + [4] * EC:
            if rem == 0:
                break
            s = min(want, rem)
            SCS.append(s)
            rem -= s
        NSC = len(SCS)
        SOFF = [sum(SCS[:i]) for i in range(NSC)]
        LAG = 1  # V/Q run one super-chunk behind K (K first in each group)

        w_sb = consts.tile([P, EC * 3 * H], F16, tag="w_all")

        def w_slice(sc):  # DMA the W chunks for super-chunk sc
            lo, hi = SOFF[sc] * 3 * H, (SOFF[sc] + SCS[sc]) * 3 * H
            nc.sync.dma_start(w_sb[:, lo:hi], w_all[:, lo:hi])

        def wk_chunk(e):
            return w_sb[:, (3 * e) * H : (3 * e + 1) * H]

        def wv_chunk(e):
            return w_sb[:, (3 * e + 1) * H : (3 * e + 2) * H]

        def wq_chunk(e):
            return w_sb[:, (3 * e + 2) * H : (3 * e + 3) * H]

        # first slice on the scalar queue so it lands in parallel with pe_t(0)
        lo0, hi0 = 0, (SOFF[0] + SCS[0]) * 3 * H
        nc.scalar.dma_start(w_sb[:, lo0:hi0], w_all[:, lo0:hi0])
        k_ps = [pp.tile([P, FD], F32, tag="pp", name=f"k_ps{i}") for i in range(NT)]
        v_ps = [pp.tile([P, FD], F32, tag="pp", name=f"v_ps{i}") for i in range(NT)]
        q_ps = [pp.tile([P, FD], F32, tag="pp", name=f"q_ps{i}") for i in range(NT)]

        def proj_mms(ps, w_chunk, t, sc):
            for c in range(SCS[sc]):
                e = SOFF[sc] + c
                for i in range(NT):
                    nc.tensor.matmul(
                        ps[i][:],
                        w_chunk(e),
                        t[:, c, i * FD : (i + 1) * FD],
                        start=(e == 0),
                        stop=(e == EC - 1),
                    )

        # ---- phase 1: projections. K is consumed at peT-stream pace (so its
        # AllGather fires early and completes under the V/Q tail); V/Q run a
        # couple of super-chunks behind on the second DMA queue.
        x_tiles = {}
        for sc in range(NSC):
            n = SCS[sc]
            off = SOFF[sc]
            pe_t = pe_pool.tile([P, n, TB], F16, tag="pe", name=f"pe_t{sc}")
            nc.sync.dma_start(
                pe_t[:],
                peT[off * P : (off + n) * P, :].rearrange("(c p) t -> p c t", p=P),
            )
            x_t = x_pool.tile([P, n, TB], F16, tag="xs", name=f"x_t{sc}")
            nc.scalar.dma_start(
                x_t[:],
                xT[off * P : (off + n) * P, :].rearrange("(c p) t -> p c t", p=P),
            )
            x_tiles[sc] = x_t
            if sc + 1 < NSC:
                w_slice(sc + 1)
            proj_mms(k_ps, wk_chunk, pe_t, sc)
            if sc >= LAG:
                proj_mms(v_ps, wv_chunk, x_tiles[sc - LAG], sc - LAG)
                proj_mms(q_ps, wq_chunk, x_tiles[sc - LAG], sc - LAG)

        # ---- K done: kick its AllGather while V/Q still run. High priority:
        # the second readback otherwise lands ~12us after K finishes and gates
        # the r=1 half of attention pass 1.
        with tc.high_priority():
            kT_loc = big.tile([P, TB], F16, tag="kT_loc")
            for i in range(NT):
                nc.vector.tensor_copy(kT_loc[:, i * FD : (i + 1) * FD], k_ps[i][:])
            nc.sync.dma_start(cc_in[:KV].rearrange("(h t) -> h t", t=TB), kT_loc[:])
            if mock_cc:
                nc.sync.dma_start(cc_k_out[0], cc_in[:KV])
                nc.sync.dma_start(cc_k_out[1], cc_in[:KV])
            else:
                nc.gpsimd.collective_compute(
                    "AllGather",
                    mybir.AluOpType.bypass,
                    replica_groups=[[0, 1], [2, 3], [4, 5], [6, 7]],
                    ins=[cc_in[:KV]],
                    outs=[cc_k_out[:]],
                )
            kT_sb = big.tile([P, 2 * TB], F16, tag="kT")
            for r in range(2):
                nc.sync.dma_start(
                    kT_sb[:, r * TB : (r + 1) * TB],
                    cc_k_out[r].rearrange("(h t) -> h t", t=TB),
                )

        # boundary-mask primitives (small) on the sync queue
        qm_sb = consts.tile([P, 2 * P], F16, tag="qm")
        nc.scalar.dma_start(qm_sb[:], qmask)

        # tail of the V/Q pipeline; V finishes first so its gather fires early
        for sc in range(max(NSC - LAG, 0), NSC - 1):
            proj_mms(v_ps, wv_chunk, x_tiles[sc], sc)
            proj_mms(q_ps, wq_chunk, x_tiles[sc], sc)
        proj_mms(v_ps, wv_chunk, x_tiles[NSC - 1], NSC - 1)

        qT_sb = big.tile([P, TB], F16, tag="qT")

        # ---- V done: transpose to natural layout, gather ----
        vT_loc = big.tile([P, TB], F16, tag="vT_loc")
        for i in range(NT):
            nc.vector.tensor_copy(vT_loc[:, i * FD : (i + 1) * FD], v_ps[i][:])
        v_loc = big.tile([P, NB, H], F16, tag="v_loc")
        for c in range(NB):
            t_ps = tr_ps_pool.tile([P, P], F16, tag="tr")
            nc.tensor.transpose(t_ps[:], vT_loc[:, c * P : (c + 1) * P], ident[:])
            nc.vector.tensor_copy(v_loc[:, c, :], t_ps[:])
        proj_mms(q_ps, wq_chunk, x_tiles[NSC - 1], NSC - 1)
        for i in range(NT):
            nc.vector.tensor_copy(qT_sb[:, i * FD : (i + 1) * FD], q_ps[i][:])
        nc.sync.dma_start(
            cc_in[KV:].rearrange("(c p h) -> p c h", p=P, h=H), v_loc[:]
        )
        if mock_cc:
            nc.sync.dma_start(cc_v_out[0], cc_in[KV:])
            nc.sync.dma_start(cc_v_out[1], cc_in[KV:])
        else:
            nc.gpsimd.collective_compute(
                "AllGather",
                mybir.AluOpType.bypass,
                replica_groups=[[0, 1], [2, 3], [4, 5], [6, 7]],
                ins=[cc_in[KV:]],
                outs=[cc_v_out[:]],
            )
        v_sb = big.tile([P, 2 * NB, H], F16, tag="v")
        for r in range(2):
            nc.sync.dma_start(
                v_sb[:, r * NB : (r + 1) * NB, :],
                cc_v_out[r].rearrange("(c p h) -> p c h", p=P, h=H),
            )


        # ---- phase 3: attention ----
        proj_ctx.close()  # release projection PSUM banks
        sT_pool = ctx.enter_context(tc.tile_pool(name="sTp", bufs=4, space="PSUM"))
        o_pool = ctx.enter_context(tc.tile_pool(name="op", bufs=2, space="PSUM"))
        d_pool = ctx.enter_context(tc.tile_pool(name="dp", bufs=2, space="PSUM"))
        # kslot-outer interleave: both query groups process a key chunk
        # back-to-back so consecutive MM1s (and MM3s) share one Ldweights,
        # and each group's denominator still accumulates independently.
        o_ps = {}
        d_ps = {}
        e_tiles = {g: [] for g in range(NQG)}
        pend = {g: None for g in range(NQG)}
        idx = {g: 0 for g in range(NQG)}
        nk = {g: QGB * (g + 1) for g in range(NQG)}
        for g in range(NQG):
            o_ps[g] = o_pool.tile([P, QG], F32, tag="o", name=f"o_ps{g}")
            d_ps[g] = d_pool.tile([1, QG], F32, tag="d", name=f"d_ps{g}")

        def flush_mm2(g, last):
            kslot, eT, col0, first = pend[g]
            nc.tensor.matmul(
                d_ps[g][:, col0:], ones_col[:], eT[:, col0:],
                start=first, stop=last,
            )

        for r in range(2):
            for c in range(NB):
                for g in range(NQG):
                    if c >= nk[g]:
                        continue
                    kslot = r * NB + c
                    col0 = (c - QGB * g) * P if c >= QGB * g else 0
                    sT = sT_pool.tile([P, QG], F32, tag="sT",
                                      name=f"sT_{g}_{r}_{c}")
                    nc.tensor.matmul(
                        sT[:, col0:],
                        kT_sb[:, kslot * P : (kslot + 1) * P],
                        qT_sb[:, g * QG + col0 : (g + 1) * QG],
                        start=True,
                        stop=True,
                    )
                    if pend[g] is not None:
                        flush_mm2(g, False)
                    eT = e_pool.tile([P, QG], F16, tag="eT", name=f"eT_{g}_{r}_{c}")
                    if c >= QGB * g:
                        nc.scalar.activation(
                            eT[:, col0:], sT[:, col0:],
                            mybir.ActivationFunctionType.Exp, scale=scale,
                        )
                        nc.vector.tensor_mul(
                            eT[:, col0 : col0 + P],
                            eT[:, col0 : col0 + P],
                            qm_sb[:, r * P : (r + 1) * P],
                        )
                    else:
                        nc.scalar.activation(
                            eT[:], sT[:], mybir.ActivationFunctionType.Exp,
                            scale=scale,
                        )
                    pend[g] = (kslot, eT, col0, idx[g] == 0)
                    e_tiles[g].append((kslot, eT, col0))
                    idx[g] += 1
        for g in range(NQG):
            flush_mm2(g, True)

        for g in range(NQG):
            last_idx = 2 * nk[g] - 1
            for i, (kslot, eT, col0) in enumerate(e_tiles[g]):
                nc.tensor.matmul(
                    o_ps[g][:, col0:], v_sb[:, kslot, :], eT[:, col0:],
                    start=(i == 0), stop=(i == last_idx),
                )

            rec = sm.tile([1, QG], F32, tag="rec", name=f"rec{g}")
            nc.vector.reciprocal(rec[:], d_ps[g][:])
            bc_ps = sT_pool.tile([P, QG], F32, tag="sT", name=f"bc_ps{g}")
            nc.tensor.matmul(bc_ps[:], ones_row[:], rec[:], start=True, stop=True)
            bc_sb = sm.tile([P, QG], F32, tag="bcs", name=f"bc_sb{g}")
            nc.vector.tensor_copy(bc_sb[:], bc_ps[:])
            oT = sm.tile([P, QG], F32, tag="oT", name=f"oT{g}")
            nc.vector.tensor_mul(oT[:], o_ps[g][:], bc_sb[:])
            nc.sync.dma_start(outT[:, g * QG : (g + 1) * QG], oT[:])

    return nc


def _core_rows(cfg: Cfg, h: int) -> np.ndarray:
    j = np.arange(cfg.TB)
    return ((j // P) * 2 + h) * P + (j % P)


def _w_layout(cfg: Cfg, Wk, Wv, Wq) -> np.ndarray:
    # each W [H, E] -> [P, EC, H] with [p, e, h] = W[h, e*P + p]; interleave
    # per chunk as [wk_e | wv_e | wq_e] so one DMA slice covers a chunk range
    def lay(W):
        return W.T.reshape(cfg.EC, P, cfg.H).transpose(1, 0, 2)

    out = np.empty((P, cfg.EC, 3, cfg.H), np.float32)
    out[:, :, 0] = lay(Wk)
    out[:, :, 1] = lay(Wv)
    out[:, :, 2] = lay(Wq)
    return np.ascontiguousarray(
        out.reshape(P, cfg.EC * 3 * cfg.H)
    ).astype(np.float16)


def _masks(cfg: Cfg, h: int) -> np.ndarray:
    # boundary primitives [P, 2P]: cols [:P] apply to own-rank-0 diagonal
    # tiles, cols [P:] to rank-1 diagonal tiles (see build() docstring)
    kt = np.arange(P)[:, None]
    qt = np.arange(P)[None, :]
    tril = (kt <= qt).astype(np.float16)
    zeros = np.zeros((P, P), np.float16)
    ones = np.ones((P, P), np.float16)
    b0 = tril if h == 0 else ones
    b1 = zeros if h == 0 else tril
    return np.concatenate([b0, b1], axis=1)


def shard_inputs(cfg: Cfg, x, pos_emb, Wq, Wk, Wv):
    x = np.asarray(x, dtype=np.float32)
    pos_emb = np.asarray(pos_emb, dtype=np.float32)
    w_l = _w_layout(cfg, np.asarray(Wk, np.float32), np.asarray(Wv, np.float32),
                    np.asarray(Wq, np.float32))
    masks = [_masks(cfg, h) for h in range(2)]
    in_maps = []
    for core in range(8):
        b, h = core // 2, core % 2
        rows = _core_rows(cfg, h)
        in_maps.append(
            {
                "xT": np.ascontiguousarray(x[b][rows].T).astype(np.float16),
                "peT": np.ascontiguousarray(pos_emb[b][rows].T).astype(np.float16),
                "w_all": w_l,
                "qmask": masks[h],
            }
        )
    return in_maps


def unshard(cfg: Cfg, results) -> np.ndarray:
    out = np.empty((cfg.B, cfg.T, cfg.H), np.float32)
    for core in range(8):
        b, h = core // 2, core % 2
        rows = _core_rows(cfg, h)
        out[b][rows] = results[core]["outT"].T
    return out


_NC_CACHE = {}


def _get_nc(cfg: Cfg) -> bass.Bass:
    # built once per process; _split_sync_waits is applied here (HW path only —
    # the injected carriers confuse CoreSim, which never sees walrus anyway)
    if cfg not in _NC_CACHE:
        nc = build(cfg)
        _dedupe_ldweights(nc)
        _split_sync_waits(nc)
        _NC_CACHE[cfg] = nc
    return _NC_CACHE[cfg]


def kernel(x, pos_emb, Wq, Wk, Wv) -> np.ndarray:
    from concourse.bass_utils import run_bass_kernel_spmd

    cfg = FULL
    nc = _get_nc(cfg)
    in_maps = shard_inputs(cfg, x, pos_emb, Wq, Wk, Wv)
    res = run_bass_kernel_spmd(nc, in_maps, list(range(8)))
    return unshard(cfg, res.results)

